# revision 11
# baseline (speedup 1.0000x reference)
"""AWQ W4A16 linear kernel for Trainium2 (8 NeuronCores, tensor-parallel over N).

out = x @ dequant(qweight, scales, qzeros) + bias
  x:       [8192, 4096]  bf16 (replicated)
  qweight: [512, 14336]  int32 (4-bit packed along K; column-sharded)
  scales:  [64, 14336]   bf16 (column-sharded)
  qzeros:  [64, 1792]    int32 (4-bit packed along N; column-sharded)
  bias:    [14336]       bf16 (column-sharded)
  out:     [8192, 14336] bf16 (gathered from per-core [8192, 1792] shards)
"""
import numpy as np
import ml_dtypes

P = 128
N_CORES = 8
M_FULL, K_FULL, N_FULL, GROUP = 8192, 4096, 14336, 64


def build_nc(M=M_FULL, K=K_FULL, NS=N_FULL // N_CORES, m_chunk=256, n_free=448):
    import concourse.bass as bass  # noqa: F401
    import concourse.mybir as mybir
    import concourse.tile as tile
    from concourse import bacc

    KT = K // P               # k-tiles of 128 rows
    QROWS_T = P // 8          # qweight rows per k-tile (16)
    NG = K // GROUP           # quantization groups (64)
    NQ = NS // 8              # packed qzeros columns
    NJ = NS // n_free         # output column chunks per psum pass
    MSUB = m_chunk // P       # m-subtiles per chunk
    assert M % m_chunk == 0 and NS % n_free == 0 and K % (GROUP * 2) == 0

    nc = bacc.Bacc("TRN2", target_bir_lowering=False, debug=False)
    dt = mybir.dt
    x = nc.dram_tensor("x", [M, K], dt.bfloat16, kind="ExternalInput")
    qw = nc.dram_tensor("qweight", [K // 8, NS], dt.int32, kind="ExternalInput")
    sc = nc.dram_tensor("scales", [NG, NS], dt.bfloat16, kind="ExternalInput")
    qz = nc.dram_tensor("qzeros", [NG, NQ], dt.int32, kind="ExternalInput")
    bias = nc.dram_tensor("bias", [NS], dt.bfloat16, kind="ExternalInput")
    out = nc.dram_tensor("out", [M, NS], dt.bfloat16, kind="ExternalOutput")

    # constants
    shift_np = (4 * (np.arange(P) % 8)).astype(np.int32).reshape(P, 1)
    shift_dram = nc.inline_tensor(shift_np, name="shiftc")
    # one-hot selector: sel[g, s, p] = 1 iff g == 2s + p//64, so
    # sel[:, s, :].T @ rows gives rows (2s, 2s+1) replicated over partition halves
    sel_np = np.zeros((NG, KT, P), np.float32)
    gg = 2 * np.arange(KT)[:, None] + np.arange(P)[None, :] // GROUP  # [KT, P]
    for s in range(KT):
        sel_np[gg[s], s, np.arange(P)] = 1.0
    sel_dram = nc.inline_tensor(sel_np.astype(ml_dtypes.bfloat16), name="selbf")
    ones_dram = nc.inline_tensor(np.ones((1, P), ml_dtypes.bfloat16), name="ones1")

    with tile.TileContext(nc) as tc:
        with tc.tile_pool(name="wres", bufs=1) as wres_pool, \
             tc.tile_pool(name="const", bufs=1) as cpool:
            w_res = wres_pool.tile([P, KT, NS], dt.bfloat16)
            bias_rep = cpool.tile([P, NS], dt.bfloat16)
            shift_col = cpool.tile([P, 1], dt.int32)
            ones_bf = cpool.tile([1, P], dt.bfloat16)
            nc.scalar.dma_start(shift_col[:], shift_dram[:])
            nc.scalar.dma_start(ones_bf[:], ones_dram[:])

            # ---------------- prologue: dequantize W into SBUF ----------------
            with tc.tile_pool(name="prol", bufs=1) as prol, \
                 tc.tile_pool(name="prol2", bufs=2) as prol2, \
                 tc.tile_pool(name="pps", bufs=1, space="PSUM") as pps:
                scales_sb = prol.tile([NG, NS], dt.bfloat16)
                nc.scalar.dma_start(scales_sb[:], sc[:])
                qz_sb = prol.tile([NG, NQ], dt.int32)
                nc.scalar.dma_start(qz_sb[:], qz[:])
                bias_sb = prol.tile([1, NS], dt.bfloat16)
                nc.scalar.dma_start(bias_sb[:], bias[None, :])
                sel_bf = prol.tile([NG, KT, P], dt.bfloat16)
                nc.scalar.dma_start(sel_bf[:], sel_dram[:])

                # unpack zeros along the free dim: z[g, 8c+j] = (qz[g, c] >> 4j) & 15
                zfull_i = prol.tile([NG, NS], dt.int32)
                zview = zfull_i.rearrange("g (c j) -> g c j", j=8)
                for j in range(8):
                    nc.vector.tensor_scalar(
                        zview[:, :, j], qz_sb[:], 4 * j, 15,
                        mybir.AluOpType.logical_shift_right, mybir.AluOpType.bitwise_and,
                    )
                zfull = prol.tile([NG, NS], dt.bfloat16)
                nc.vector.tensor_copy(zfull[:], zfull_i[:])

                # bias replicated across partitions (ones outer product)
                for c in range(0, NS, 512):
                    w = min(512, NS - c)
                    bps = pps.tile([P, 512], dt.float32, tag="biasps")
                    nc.tensor.matmul(bps[:, :w], ones_bf[:], bias_sb[:, c:c + w],
                                     start=True, stop=True)
                    nc.vector.tensor_copy(bias_rep[:, c:c + w], bps[:, :w])

                for s in range(KT):
                    # load qweight rows [16s, 16s+16) with 8x partition replication:
                    # partition p holds qweight row 16s + p//8
                    q_rep = prol2.tile([P, NS], dt.int32, tag="qrep")
                    qv = q_rep.rearrange("(a b) n -> a b n", b=8)
                    for b in range(8):
                        nc.scalar.dma_start(qv[:, b, :], qw[QROWS_T * s:QROWS_T * (s + 1), :])
                    # w4 = (q >> 4*(p%8)) & 15
                    shifted_i = prol2.tile([P, NS], dt.int32, tag="shifted_i")
                    nc.vector.tensor_scalar(
                        shifted_i[:], q_rep[:], shift_col[:], 15,
                        mybir.AluOpType.logical_shift_right, mybir.AluOpType.bitwise_and,
                    )
                    shifted = prol2.tile([P, NS], dt.float32, tag="shifted")
                    nc.vector.tensor_copy(shifted[:], shifted_i[:])
                    # replicate scales/zeros rows (2s, 2s+1) to partition halves via PE,
                    # in column halves to fit PSUM; then W = (w4 - z) * s
                    half = NS // 2
                    for h in range(2):
                        h0 = h * half
                        srep = pps.tile([P, half], dt.float32, tag="srep", name="srep")
                        zrep = pps.tile([P, half], dt.float32, tag="zrep", name="zrep")
                        for c in range(h0, h0 + half, 512):
                            w = min(512, h0 + half - c)
                            nc.tensor.matmul(srep[:, c - h0:c - h0 + w], sel_bf[:, s, :],
                                             scales_sb[:, c:c + w], start=True, stop=True)
                            nc.tensor.matmul(zrep[:, c - h0:c - h0 + w], sel_bf[:, s, :],
                                             zfull[:, c:c + w], start=True, stop=True)
                        t1 = prol2.tile([P, half], dt.float32, tag="t1", name="t1")
                        nc.vector.tensor_tensor(t1[:], shifted[:, h0:h0 + half], zrep[:],
                                                mybir.AluOpType.subtract)
                        nc.vector.tensor_tensor(w_res[:, s, h0:h0 + half], t1[:], srep[:],
                                                mybir.AluOpType.mult)

            # ---------------- main loop: out = x @ W + bias ----------------
            with tc.tile_pool(name="xt", bufs=2) as xtp, \
                 tc.tile_pool(name="stage", bufs=3) as stp, \
                 tc.tile_pool(name="mps", bufs=2, space="PSUM") as mps:
                for ci in range(M // m_chunk):
                    m0 = ci * m_chunk
                    xt = xtp.tile([P, KT, m_chunk], dt.bfloat16)
                    for s in range(KT):
                        nc.sync.dma_start(xt[:, s, :], x[m0:m0 + m_chunk, P * s:P * (s + 1)],
                                          transpose=True)
                    for i in range(MSUB):
                        psums = [
                            mps.tile([P, n_free], dt.float32, tag=f"acc{j}", name=f"acc{j}")
                            for j in range(NJ)
                        ]
                        for s in range(KT):
                            lhsT = xt[:, s, P * i:P * (i + 1)]
                            for j in range(NJ):
                                nc.tensor.matmul(
                                    psums[j], lhsT, w_res[:, s, n_free * j:n_free * (j + 1)],
                                    start=(s == 0), stop=(s == KT - 1),
                                )
                        stage = stp.tile([P, NS], dt.bfloat16)
                        for j in range(NJ):
                            nc.vector.tensor_tensor(
                                stage[:, n_free * j:n_free * (j + 1)], psums[j],
                                bias_rep[:, n_free * j:n_free * (j + 1)], mybir.AluOpType.add,
                            )
                        nc.scalar.dma_start(out[m0 + P * i:m0 + P * (i + 1), :], stage[:])
    nc.compile()
    return nc


def _shard_inputs(inputs):
    ns = N_FULL // N_CORES
    nq = ns // 8
    x = np.asarray(inputs["x"])
    qw = np.asarray(inputs["qweight"])
    sc = np.asarray(inputs["scales"])
    qz = np.asarray(inputs["qzeros"])
    bias = np.asarray(inputs["bias"])
    in_maps = []
    for c in range(N_CORES):
        in_maps.append({
            "x": x,
            "qweight": np.ascontiguousarray(qw[:, c * ns:(c + 1) * ns]),
            "scales": np.ascontiguousarray(sc[:, c * ns:(c + 1) * ns]),
            "qzeros": np.ascontiguousarray(qz[:, c * nq:(c + 1) * nq]),
            "bias": np.ascontiguousarray(bias[c * ns:(c + 1) * ns]),
        })
    return in_maps


_NC_CACHE = {}


def _get_nc():
    if "nc" not in _NC_CACHE:
        _NC_CACHE["nc"] = build_nc()
    return _NC_CACHE["nc"]


def kernel(**inputs) -> np.ndarray:
    from concourse.bass_utils import run_bass_kernel_spmd

    nc = _get_nc()
    in_maps = _shard_inputs(inputs)
    res = run_bass_kernel_spmd(nc, in_maps, core_ids=list(range(N_CORES)))
    return np.concatenate([res.results[c]["out"] for c in range(N_CORES)], axis=1)
